# revision 6
# baseline (speedup 1.0000x reference)
"""Mixture-of-Experts (B=4, S=2048, D=1024, F=4096, E=8, top-2) on 8 trn2 NeuronCores.

Strategy: expert parallelism, one expert per core.
  - Host: gate (softmax + top-2 + renorm) in float64, dispatch (gather) tokens
    per expert, pad to a common capacity C, transpose to [D, C] so the
    contraction dim lands on SBUF partitions with zero on-device transposes.
  - Device (SPMD, identical program, per-core data): y^T = W2^T @ gelu(W1^T @ x^T + b1) + b2
    with both weights resident in SBUF as bf16 and tokens streamed in chunks
    of 512. PSUM accumulates over the contraction (D resp. F) in fp32.
  - Host: combine with the gate weights (y *= cw) and scatter-add back into
    the [B*S, D] output. Token index sets are unique per expert, so fancy-index
    add per expert is race-free.
"""

import copy
import sys

import numpy as np

for _p in ("/opt/trn_rl_repo", "/opt/pypackages"):
    if _p not in sys.path:
        sys.path.append(_p)

import ml_dtypes

B, S, D = 4, 2048, 1024
F = 4 * D
E = 8
TOP_K = 2
P = 128
C_CHUNK = 512

# test-harness hooks (left off for grading)
TRACE = False
LAST_RESULTS = None

_compiled = {}


def _split_drain_waits(nc, max_waits=1):
    """This walrus build rejects instructions carrying more than one sync
    wait ("Too many sync wait commands"). Keep one wait on the instruction and
    move the excess onto NoOps inserted right before it on the same engine
    (engines are in-order, so blocking semantics are identical). Updates stay
    on the original instruction — moving them to a trailing NoOp could signal
    before the op's writes land."""
    import concourse.mybir as mybir

    m = nc.m
    new_module = copy.replace(m, functions=[])
    for function in m.functions:
        new_function = copy.replace(function, blocks=[])
        new_function.set_allocations_from_list(function.allocations)
        for block in function.blocks:
            out = []
            for inst in block.instructions:
                si = getattr(inst, "sync_info", None)
                on_wait = list(si.on_wait) if si is not None and si.on_wait else []
                if len(on_wait) > max_waits:
                    engine = getattr(inst, "engine", None)
                    extra, keep = on_wait[max_waits:], on_wait[:max_waits]
                    for j, w in enumerate(extra):
                        out.append(
                            mybir.InstNoOp(
                                name=f"{inst.name}-w{j}",
                                engine=engine,
                                sync_info=mybir.SyncInfo(on_wait=[w], on_update=[]),
                                bass_nofuse=True,
                            )
                        )
                    inst.sync_info = mybir.SyncInfo(
                        on_wait=keep,
                        on_update=list(si.on_update) if si.on_update else [],
                    )
                out.append(inst)
            new_function.blocks.append(copy.replace(block, instructions=out))
        new_module.functions.append(new_function)
    nc.m = new_module
    return nc


def _build_nc(C):
    import concourse.bass as bass
    import concourse.mybir as mybir
    from concourse.tile import TileContext

    fp32 = mybir.dt.float32
    bf16 = mybir.dt.bfloat16
    AF = mybir.ActivationFunctionType

    KO = D // P           # 8  k-subtiles for the first matmul
    FT = F // P           # 32 f-tiles (partition tiles of h)
    DT = D // P           # 8  d-tiles (partition tiles of y)

    nc = bass.Bass()
    xT = nc.declare_dram_parameter("xT", [D, C], bf16, isOutput=False)
    w1 = nc.declare_dram_parameter("w1", [D, F], bf16, isOutput=False)
    w2 = nc.declare_dram_parameter("w2", [F, D], bf16, isOutput=False)
    b1 = nc.declare_dram_parameter("b1", [F], fp32, isOutput=False)
    b2 = nc.declare_dram_parameter("b2", [D], fp32, isOutput=False)
    yT = nc.declare_dram_parameter("yT", [D, C], fp32, isOutput=True)

    xT_r = xT.rearrange("(ko ki) c -> ki ko c", ki=P)
    w1_r = w1.rearrange("(ko ki) f -> ki ko f", ki=P)
    w2_r = w2.rearrange("(fo fi) d -> fi fo d", fi=P)
    yT_r = yT.rearrange("(do di) c -> di do c", di=P)

    chunks = []
    c0 = 0
    while c0 < C:
        chunks.append((c0, min(C_CHUNK, C - c0)))
        c0 += C_CHUNK

    with TileContext(nc) as tc:
        with (
            tc.tile_pool(name="wpool", bufs=1) as wpool,
            tc.tile_pool(name="xpool", bufs=2) as xpool,
            tc.tile_pool(name="hpool", bufs=1) as hpool,
            tc.tile_pool(name="ypool", bufs=4) as ypool,
            tc.tile_pool(name="hpsum", bufs=3, space="PSUM") as hpsum,
            tc.tile_pool(name="ypsum", bufs=3, space="PSUM") as ypsum,
        ):
            # Resident weights / biases (one-time loads, split for DMA overlap)
            w1_sb = wpool.tile([P, KO, F], bf16)
            for ko in range(KO):
                nc.sync.dma_start(w1_sb[:, ko, :], w1_r[:, ko, :])
            w2_sb = wpool.tile([P, FT, D], bf16)
            for fo in range(FT):
                nc.sync.dma_start(w2_sb[:, fo, :], w2_r[:, fo, :])
            b1_sb = wpool.tile([P, FT], fp32)
            nc.sync.dma_start(b1_sb[:], b1.rearrange("(fo fi) -> fi fo", fi=P))
            b2_sb = wpool.tile([P, DT], fp32)
            nc.sync.dma_start(b2_sb[:], b2.rearrange("(do di) -> di do", di=P))

            for c0, cn in chunks:
                x_sb = xpool.tile([P, KO, C_CHUNK], bf16, tag="x")
                nc.sync.dma_start(x_sb[:, :, :cn], xT_r[:, :, c0:c0 + cn])

                h_sb = hpool.tile([P, FT, C_CHUNK], bf16, tag="h")
                for ft in range(FT):
                    h_ps = hpsum.tile([P, C_CHUNK], fp32, tag="hps")
                    for ko in range(KO):
                        nc.tensor.matmul(
                            h_ps[:, :cn],
                            w1_sb[:, ko, ft * P:(ft + 1) * P],
                            x_sb[:, ko, :cn],
                            start=(ko == 0),
                            stop=(ko == KO - 1),
                        )
                    # gelu(mm + b1) fused on ScalarE, cast to bf16 on write
                    nc.scalar.activation(
                        h_sb[:, ft, :cn], h_ps[:, :cn], AF.Gelu,
                        bias=b1_sb[:, ft:ft + 1],
                    )

                for dt_ in range(DT):
                    y_ps = ypsum.tile([P, C_CHUNK], fp32, tag="yps")
                    for fo in range(FT):
                        nc.tensor.matmul(
                            y_ps[:, :cn],
                            w2_sb[:, fo, dt_ * P:(dt_ + 1) * P],
                            h_sb[:, fo, :cn],
                            start=(fo == 0),
                            stop=(fo == FT - 1),
                        )
                    y_sb = ypool.tile([P, C_CHUNK], fp32, tag="y")
                    nc.vector.tensor_scalar_add(
                        y_sb[:, :cn], y_ps[:, :cn], b2_sb[:, dt_:dt_ + 1]
                    )
                    nc.sync.dma_start(yT_r[:, dt_, c0:c0 + cn], y_sb[:, :cn])

    return _split_drain_waits(nc)


def _to_bf16(a):
    """Fast float32 -> bfloat16 with round-to-nearest-even via bit ops."""
    a = np.ascontiguousarray(a, dtype=np.float32)
    u = a.view(np.uint32)
    r = ((u + 0x7FFF + ((u >> 16) & 1)) >> 16).astype(np.uint16)
    return r.view(ml_dtypes.bfloat16)


def kernel(hidden_states, Wg, bg, W1, b1, W2, b2):
    from concourse import bass_utils

    hs = np.ascontiguousarray(hidden_states, dtype=np.float32).reshape(B * S, D)

    # ---- Gate on host (float64): softmax over experts, top-2, renormalize
    logits = hs.astype(np.float64) @ np.asarray(Wg, np.float64).T
    logits += np.asarray(bg, np.float64)
    logits -= logits.max(axis=-1, keepdims=True)
    p = np.exp(logits)
    p /= p.sum(axis=-1, keepdims=True)

    i1 = p.argmax(axis=-1)
    rows = np.arange(B * S)
    p1 = p[rows, i1]
    pm = p.copy()
    pm[rows, i1] = -1.0
    i2 = pm.argmax(axis=-1)
    p2 = p[rows, i2]
    denom = p1 + p2
    g1 = (p1 / denom).astype(np.float32)
    g2 = (p2 / denom).astype(np.float32)

    # ---- Dispatch: token ids + combine weights per expert
    ids, cws = [], []
    for e in range(E):
        m1 = np.nonzero(i1 == e)[0]
        m2 = np.nonzero(i2 == e)[0]
        ids.append(np.concatenate([m1, m2]))
        cws.append(np.concatenate([g1[m1], g2[m2]]))
    max_cnt = max(len(x) for x in ids)
    C = max(P, -(-max_cnt // P) * P)

    if C not in _compiled:
        _compiled[C] = _build_nc(C)
    nc = _compiled[C]

    in_maps = []
    for e in range(E):
        xT = np.zeros((D, C), dtype=ml_dtypes.bfloat16)
        cnt = len(ids[e])
        xT[:, :cnt] = _to_bf16(hs[ids[e]]).T
        in_maps.append({
            "xT": xT,
            "w1": _to_bf16(W1[e]),
            "w2": _to_bf16(W2[e]),
            "b1": np.ascontiguousarray(b1[e], dtype=np.float32),
            "b2": np.ascontiguousarray(b2[e], dtype=np.float32),
        })

    kwargs = {}
    if TRACE:
        import os as _os
        kwargs = dict(trace=True, trace_cores=list(range(E)))
        if _os.environ.get("MOE_TRACE_DIR"):
            _os.makedirs(_os.environ["MOE_TRACE_DIR"], exist_ok=True)
            kwargs["tmpdir"] = _os.environ["MOE_TRACE_DIR"]
    res = bass_utils.run_bass_kernel_spmd(nc, in_maps, list(range(E)), **kwargs)
    global LAST_RESULTS
    LAST_RESULTS = res

    out = np.zeros((B * S, D), dtype=np.float32)
    for e in range(E):
        cnt = len(ids[e])
        yT = res.results[e]["yT"]
        out[ids[e]] += cws[e][:, None] * yT[:, :cnt].T
    return out.reshape(B, S, D)


# revision 7
# speedup vs baseline: 1.0630x; 1.0630x over previous
"""Mixture-of-Experts (B=4, S=2048, D=1024, F=4096, E=8, top-2) on 8 trn2 NeuronCores.

Strategy: expert parallelism, one expert per core.
  - Host: gate (softmax + top-2 + renorm) in float64, dispatch (gather) tokens
    per expert, pad to a common capacity C, transpose to [D, C] so the
    contraction dim lands on SBUF partitions with zero on-device transposes.
  - Device (SPMD, identical program, per-core data): y^T = W2^T @ gelu(W1^T @ x^T + b1) + b2
    with both weights resident in SBUF as bf16 and tokens streamed in chunks
    of 512. PSUM accumulates over the contraction (D resp. F) in fp32.
  - Host: combine with the gate weights (y *= cw) and scatter-add back into
    the [B*S, D] output. Token index sets are unique per expert, so fancy-index
    add per expert is race-free.
"""

import copy
import sys

import numpy as np

for _p in ("/opt/trn_rl_repo", "/opt/pypackages"):
    if _p not in sys.path:
        sys.path.append(_p)

import ml_dtypes

B, S, D = 4, 2048, 1024
F = 4 * D
E = 8
TOP_K = 2
P = 128
C_CHUNK = 512

# test-harness hooks (left off for grading)
TRACE = False
LAST_RESULTS = None

_compiled = {}


def _split_drain_waits(nc, max_waits=1):
    """This walrus build rejects instructions carrying more than one sync
    wait ("Too many sync wait commands"). Keep one wait on the instruction and
    move the excess onto NoOps inserted right before it on the same engine
    (engines are in-order, so blocking semantics are identical). Updates stay
    on the original instruction — moving them to a trailing NoOp could signal
    before the op's writes land."""
    import concourse.mybir as mybir

    m = nc.m
    new_module = copy.replace(m, functions=[])
    for function in m.functions:
        new_function = copy.replace(function, blocks=[])
        new_function.set_allocations_from_list(function.allocations)
        for block in function.blocks:
            out = []
            for inst in block.instructions:
                si = getattr(inst, "sync_info", None)
                on_wait = list(si.on_wait) if si is not None and si.on_wait else []
                if len(on_wait) > max_waits:
                    engine = getattr(inst, "engine", None)
                    extra, keep = on_wait[max_waits:], on_wait[:max_waits]
                    for j, w in enumerate(extra):
                        out.append(
                            mybir.InstNoOp(
                                name=f"{inst.name}-w{j}",
                                engine=engine,
                                sync_info=mybir.SyncInfo(on_wait=[w], on_update=[]),
                                bass_nofuse=True,
                            )
                        )
                    inst.sync_info = mybir.SyncInfo(
                        on_wait=keep,
                        on_update=list(si.on_update) if si.on_update else [],
                    )
                out.append(inst)
            new_function.blocks.append(copy.replace(block, instructions=out))
        new_module.functions.append(new_function)
    nc.m = new_module
    return nc


def _build_nc(C):
    import concourse.bass as bass
    import concourse.mybir as mybir
    from concourse.tile import TileContext

    fp32 = mybir.dt.float32
    bf16 = mybir.dt.bfloat16
    AF = mybir.ActivationFunctionType

    KO = D // P           # 8  k-subtiles for the first matmul
    FT = F // P           # 32 f-tiles (partition tiles of h)
    DT = D // P           # 8  d-tiles (partition tiles of y)

    nc = bass.Bass()
    xT = nc.declare_dram_parameter("xT", [D, C], bf16, isOutput=False)
    w1 = nc.declare_dram_parameter("w1", [D, F], bf16, isOutput=False)
    w2 = nc.declare_dram_parameter("w2", [F, D], bf16, isOutput=False)
    b1 = nc.declare_dram_parameter("b1", [F], fp32, isOutput=False)
    b2 = nc.declare_dram_parameter("b2", [D], fp32, isOutput=False)
    yT = nc.declare_dram_parameter("yT", [D, C], fp32, isOutput=True)

    xT_r = xT.rearrange("(ko ki) c -> ki ko c", ki=P)
    w1_r = w1.rearrange("(ko ki) f -> ki ko f", ki=P)
    w2_r = w2.rearrange("(fo fi) d -> fi fo d", fi=P)
    yT_r = yT.rearrange("(do di) c -> di do c", di=P)

    chunks = []
    c0 = 0
    while c0 < C:
        chunks.append((c0, min(C_CHUNK, C - c0)))
        c0 += C_CHUNK

    with TileContext(nc) as tc:
        with (
            tc.tile_pool(name="wpool", bufs=1) as wpool,
            tc.tile_pool(name="xpool", bufs=3) as xpool,
            tc.tile_pool(name="hpool", bufs=1) as hpool,
            tc.tile_pool(name="ypool", bufs=4) as ypool,
            tc.tile_pool(name="hpsum", bufs=3, space="PSUM") as hpsum,
            tc.tile_pool(name="ypsum", bufs=3, space="PSUM") as ypsum,
        ):
            # Chunk-0 activations first so their DMA isn't queued behind the
            # 16.8 MB weight load; then W1 (needed first, per-ko tiles for
            # fine-grained deps), then W2 (streams during chunk-0's mm1).
            c0_0, cn_0 = chunks[0]
            x0_sb = xpool.tile([P, KO, C_CHUNK], bf16, tag="x")
            nc.sync.dma_start(x0_sb[:, :, :cn_0], xT_r[:, :, c0_0:c0_0 + cn_0])

            w1_t = []
            for ko in range(KO):
                t = wpool.tile([P, F], bf16, tag=f"w1_{ko}")
                nc.sync.dma_start(t[:], w1_r[:, ko, :])
                w1_t.append(t)
            b1_sb = wpool.tile([P, FT], fp32)
            nc.sync.dma_start(b1_sb[:], b1.rearrange("(fo fi) -> fi fo", fi=P))
            b2_sb = wpool.tile([P, DT], fp32)
            nc.sync.dma_start(b2_sb[:], b2.rearrange("(do di) -> di do", di=P))
            w2_t = []
            for fo in range(FT):
                t = wpool.tile([P, D], bf16, tag=f"w2_{fo}")
                nc.sync.dma_start(t[:], w2_r[:, fo, :])
                w2_t.append(t)

            for ci, (c0, cn) in enumerate(chunks):
                if ci == 0:
                    x_sb = x0_sb
                else:
                    x_sb = xpool.tile([P, KO, C_CHUNK], bf16, tag="x")
                    nc.sync.dma_start(x_sb[:, :, :cn], xT_r[:, :, c0:c0 + cn])

                h_sb = hpool.tile([P, FT, C_CHUNK], bf16, tag="h")
                for ft in range(FT):
                    h_ps = hpsum.tile([P, C_CHUNK], fp32, tag="hps")
                    for ko in range(KO):
                        nc.tensor.matmul(
                            h_ps[:, :cn],
                            w1_t[ko][:, ft * P:(ft + 1) * P],
                            x_sb[:, ko, :cn],
                            start=(ko == 0),
                            stop=(ko == KO - 1),
                        )
                    # gelu(mm + b1) fused on ScalarE, cast to bf16 on write
                    nc.scalar.activation(
                        h_sb[:, ft, :cn], h_ps[:, :cn], AF.Gelu,
                        bias=b1_sb[:, ft:ft + 1],
                    )

                for dt_ in range(DT):
                    y_ps = ypsum.tile([P, C_CHUNK], fp32, tag="yps")
                    for fo in range(FT):
                        nc.tensor.matmul(
                            y_ps[:, :cn],
                            w2_t[fo][:, dt_ * P:(dt_ + 1) * P],
                            h_sb[:, fo, :cn],
                            start=(fo == 0),
                            stop=(fo == FT - 1),
                        )
                    y_sb = ypool.tile([P, C_CHUNK], fp32, tag="y")
                    nc.vector.tensor_scalar_add(
                        y_sb[:, :cn], y_ps[:, :cn], b2_sb[:, dt_:dt_ + 1]
                    )
                    nc.sync.dma_start(yT_r[:, dt_, c0:c0 + cn], y_sb[:, :cn])

    return _split_drain_waits(nc)


def _to_bf16(a):
    """Fast float32 -> bfloat16 with round-to-nearest-even via bit ops."""
    a = np.ascontiguousarray(a, dtype=np.float32)
    u = a.view(np.uint32)
    r = ((u + 0x7FFF + ((u >> 16) & 1)) >> 16).astype(np.uint16)
    return r.view(ml_dtypes.bfloat16)


def kernel(hidden_states, Wg, bg, W1, b1, W2, b2):
    from concourse import bass_utils

    hs = np.ascontiguousarray(hidden_states, dtype=np.float32).reshape(B * S, D)

    # ---- Gate on host (float64): softmax over experts, top-2, renormalize
    logits = hs.astype(np.float64) @ np.asarray(Wg, np.float64).T
    logits += np.asarray(bg, np.float64)
    logits -= logits.max(axis=-1, keepdims=True)
    p = np.exp(logits)
    p /= p.sum(axis=-1, keepdims=True)

    i1 = p.argmax(axis=-1)
    rows = np.arange(B * S)
    p1 = p[rows, i1]
    pm = p.copy()
    pm[rows, i1] = -1.0
    i2 = pm.argmax(axis=-1)
    p2 = p[rows, i2]
    denom = p1 + p2
    g1 = (p1 / denom).astype(np.float32)
    g2 = (p2 / denom).astype(np.float32)

    # ---- Dispatch: token ids + combine weights per expert
    ids, cws = [], []
    for e in range(E):
        m1 = np.nonzero(i1 == e)[0]
        m2 = np.nonzero(i2 == e)[0]
        ids.append(np.concatenate([m1, m2]))
        cws.append(np.concatenate([g1[m1], g2[m2]]))
    max_cnt = max(len(x) for x in ids)
    C = max(P, -(-max_cnt // P) * P)

    if C not in _compiled:
        _compiled[C] = _build_nc(C)
    nc = _compiled[C]

    in_maps = []
    for e in range(E):
        xT = np.zeros((D, C), dtype=ml_dtypes.bfloat16)
        cnt = len(ids[e])
        xT[:, :cnt] = _to_bf16(hs[ids[e]]).T
        in_maps.append({
            "xT": xT,
            "w1": _to_bf16(W1[e]),
            "w2": _to_bf16(W2[e]),
            "b1": np.ascontiguousarray(b1[e], dtype=np.float32),
            "b2": np.ascontiguousarray(b2[e], dtype=np.float32),
        })

    kwargs = {}
    if TRACE:
        import os as _os
        kwargs = dict(trace=True, trace_cores=list(range(E)))
        if _os.environ.get("MOE_TRACE_DIR"):
            _os.makedirs(_os.environ["MOE_TRACE_DIR"], exist_ok=True)
            kwargs["tmpdir"] = _os.environ["MOE_TRACE_DIR"]
    res = bass_utils.run_bass_kernel_spmd(nc, in_maps, list(range(E)), **kwargs)
    global LAST_RESULTS
    LAST_RESULTS = res

    out = np.zeros((B * S, D), dtype=np.float32)
    for e in range(E):
        cnt = len(ids[e])
        yT = res.results[e]["yT"]
        out[ids[e]] += cws[e][:, None] * yT[:, :cnt].T
    return out.reshape(B, S, D)
